# revision 12
# baseline (speedup 1.0000x reference)
"""Trainium2 Bass kernel for nn_Attention_59528246723073.

Reference (per batch b, channel c; x[b,c] is [S=256, T=64]):
    fs = tanh(x @ Wspect[c])            # [S]   (contract T)
    ft = tanh(x.T @ Wtemp[c])           # [T]   (contract S)
    a  = softmax_S(fs) * 100
    g  = softmax_T(ft)
    out[b,c,s,t] = x[b,c,s,t] * a[s] * g[t]

Distribution: data-parallel over batch B=32 -> 4 per core on 8 cores.

All tensors are marshaled to fp16 on the host (input cast + output upcast
are host-side numpy), so every DMA is a plain HWDGE transfer and HBM
traffic is halved vs f32.  Per-core layout: [128 part = channels, S*T
free] fp16 per local batch; all big elementwise ops are DVE fp16
tensor_tensor in the 2x_1p perf mode (innermost step 1 everywhere).

Shared-product trick: with wm[c,s,t] = Wtemp[c,s]*Wspect[c,t] built once
on chip, a single product P = x*wm serves BOTH reductions:
    fs[c,s] = (sum_t P[c,s,:]) / Wtemp[c,s]
    ft[c,t] = (sum_s P[c,:,t]) / Wspect[c,t]
which saves one full FD=16384 DVE pass per batch vs computing x*Wspect
and x*Wtemp separately.  The divides are tiny f32 ops on [C,S]/[C,T];
the exact same fp16 weight values are used in wm and in the reciprocals,
so the cancellation is clean (validated vs the f32 reference: rel err
1.05e-2 vs 0.98e-2 for the two-product scheme).
Batch 0 uses the classic two-product path so the wm build (ScalarE
replication + 2 DVE muls) fully overlaps batch 0's compute and the
kernel ramps up on the tiny Wspect load alone.

Per batch (b>0), processed in two s-halves to keep SBUF small: P-mul;
ft level-1 fold into a scratch tile (P kept intact); fs folds strided
IN-PLACE inside P (legal: ft has already consumed P); both chains
finish in f32.  Softmax skips the max-subtraction (logits are tanh
outputs in [-1,1]) and exp's fused accum_out provides the sum.
Finals: g-mul (inner-contiguous bcast) then a-mul via paired-duplicate
a2[p,2s+j]=a[p,s]; stores alternate between the two HWDGE rings; the
last batch's finals are split into eighths so the tail out-DMA exposure
is short.
"""

import numpy as np

import concourse.tile as tile
from concourse import bacc, mybir
from concourse.bass_utils import run_bass_kernel_spmd

B, C, S, T = 32, 128, 256, 64
N_CORES = 8
B_LOC = B // N_CORES
F32 = mybir.dt.float32
F16 = mybir.dt.float16
H = S * T // 2  # 8192

_NC = None


def build_nc():
    nc = bacc.Bacc("TRN2", target_bir_lowering=False, debug=False)
    x = nc.dram_tensor("x", [B_LOC, C, S, T], F16, kind="ExternalInput")
    ws = nc.dram_tensor("wspect", [C, T], F16, kind="ExternalInput")
    wt = nc.dram_tensor("wtemp", [C, S], F16, kind="ExternalInput")
    out = nc.dram_tensor("out", [B_LOC, C, S, T], F16, kind="ExternalOutput")

    AF = mybir.ActivationFunctionType
    OP = mybir.AluOpType
    AX = mybir.AxisListType

    with tile.TileContext(nc) as tc:
        with (
            tc.tile_pool(name="consts", bufs=1) as cpool,
            tc.tile_pool(name="x2", bufs=2) as x2pool,
            tc.tile_pool(name="pp", bufs=1) as ppool,
            tc.tile_pool(name="scr", bufs=2) as scrpool,
            tc.tile_pool(name="ocp", bufs=3) as ocpool,
            tc.tile_pool(name="small", bufs=1) as spool,
        ):
            # --- constants (fp16 straight from HBM via HWDGE) ---
            ws16 = cpool.tile([C, T], F16)
            nc.sync.dma_start(ws16[:], ws[:])
            wt16 = cpool.tile([C, S], F16)
            nc.scalar.dma_start(wt16[:], wt[:])
            # f32 reciprocals of the fp16 weights (for the shared-product
            # divides)
            w32 = spool.tile([C, S], F32, tag="w32")
            rws = cpool.tile([C, T], F32)
            nc.vector.tensor_copy(w32[:, 0:T], ws16[:])
            nc.vector.reciprocal(rws[:], w32[:, 0:T])
            w32b = spool.tile([C, S], F32, tag="w32b")
            rwt = cpool.tile([C, S], F32)
            nc.vector.tensor_copy(w32b[:], wt16[:])
            nc.vector.reciprocal(rwt[:], w32b[:])

            # wt_rep halves [C, 128, T]: wt broadcast along t, built in
            # graduated pieces on ScalarE (feeds batch 0's ft path and the
            # one-time wm build).
            wt_rep = [
                scrpool.tile([C, H], F16, tag="scr", name=f"wt_rep{h}")
                for h in range(2)
            ]
            wt_rep3 = [w.rearrange("p (s t) -> p s t", t=T) for w in wt_rep]
            for q in range(8):
                sq = slice((q % 4) * 32, (q % 4) * 32 + 32)
                nc.scalar.activation(
                    wt_rep3[q // 4][:, sq, :],
                    wt16[:, (q // 4) * 128 + sq.start : (q // 4) * 128 + sq.stop]
                    .unsqueeze(2)
                    .to_broadcast((C, 32, T)),
                    AF.Copy,
                )
            # wm[c, (s,t)] = wt[c,s] * ws[c,t] (one-time, 2 DVE muls that
            # overlap batch 0's loads/compute)
            wm = cpool.tile([C, S * T], F16)
            wm3 = wm.rearrange("p (s t) -> p s t", t=T)
            for h in range(2):
                nc.vector.tensor_tensor(
                    wm3[:, h * 128 : h * 128 + 128, :],
                    wt_rep3[h],
                    ws16.unsqueeze(1).to_broadcast((C, 128, T)),
                    op=OP.mult,
                )

            for b in range(B_LOC):
                X2 = x2pool.tile([C, S * T], F16, tag="X2")
                X23 = X2.rearrange("p (s t) -> p s t", t=T)
                fs = spool.tile([C, S], F32, tag="fs")
                ft = spool.tile([C, T], F32, tag="ft")
                fth = spool.tile([C, 2 * T], F16, tag="fth")
                ftf = spool.tile([C, 2 * T], F32, tag="ftf")

                if b == 0:
                    # ---- classic two-product path, graduated chunks ----
                    fstmp = ocpool.tile([C, H], F16, tag="oc", name="fstmp")
                    fttmp = ocpool.tile([C, H], F16, tag="oc", name="fttmp")
                    ft3 = fttmp.rearrange("p (s t) -> p s t", t=T)
                    chunks = (32, 32, 64, 128)
                    s0 = 0
                    for k, sc in enumerate(chunks):
                        sl = slice(s0, s0 + sc)
                        with nc.named_scope("load"):
                            for q0 in range(s0, s0 + sc, 64):
                                sq = slice(q0, min(q0 + 64, s0 + sc))
                                nc.sync.dma_start(
                                    X2[:, sq.start * T : sq.stop * T],
                                    x[b, :, sq, :],
                                )
                        xc = X23[:, sl, :]
                        # fs[:, sl] = sum_t xc * ws
                        with nc.named_scope("fs"):
                            t3 = fstmp.rearrange("p (s t) -> p s t", t=T)[
                                :, 0:sc, :
                            ]
                            nc.vector.tensor_tensor(
                                t3, xc,
                                ws16.unsqueeze(1).to_broadcast((C, sc, T)),
                                op=OP.mult,
                            )
                            w = T // 2
                            while w >= 2:
                                nc.vector.tensor_tensor(
                                    t3[:, :, 0:w], t3[:, :, 0:w],
                                    t3[:, :, w : 2 * w], op=OP.add,
                                )
                                w //= 2
                            nc.vector.reduce_sum(
                                fs[:, sl], t3[:, :, 0:2], axis=AX.X
                            )
                        # ft partial: fold xc * wt_rep over s
                        with nc.named_scope("ft"):
                            rep = wt_rep3[s0 // 128][
                                :, s0 % 128 : s0 % 128 + sc, :
                            ]
                            nc.vector.tensor_tensor(
                                ft3[:, 0:sc, :], xc, rep, op=OP.mult
                            )
                            w = sc * T // 2
                            while w >= 2 * T:
                                nc.vector.tensor_tensor(
                                    fttmp[:, 0:w], fttmp[:, 0:w],
                                    fttmp[:, w : 2 * w], op=OP.add,
                                )
                                w //= 2
                            if k == 0:
                                nc.vector.tensor_copy(
                                    ftf[:], fttmp[:, 0 : 2 * T]
                                )
                            else:
                                nc.vector.tensor_copy(
                                    fth[:], fttmp[:, 0 : 2 * T]
                                )
                                nc.vector.tensor_tensor(
                                    ftf[:], ftf[:], fth[:], op=OP.add
                                )
                        s0 += sc
                else:
                    # ---- shared-product path, two s-halves ----
                    ftw = scrpool.tile([C, H], F16, tag="scr", name=f"ftw{b}")
                    for h in range(2):
                        sl = slice(h * 128, h * 128 + 128)
                        with nc.named_scope("load"):
                            for q0 in range(h * 128, h * 128 + 128, 64):
                                nc.sync.dma_start(
                                    X2[:, q0 * T : (q0 + 64) * T],
                                    x[b, :, q0 : q0 + 64, :],
                                )
                        P = ppool.tile([C, H], F16, tag="P")
                        P3 = P.rearrange("p (s t) -> p s t", t=T)
                        with nc.named_scope("pmul"):
                            nc.vector.tensor_tensor(
                                P[:], X2[:, h * H : (h + 1) * H],
                                wm[:, h * H : (h + 1) * H], op=OP.mult,
                            )
                        # ft: level-1 fold into scratch (keeps P intact),
                        # then in-place fp16 folds down to FD=2T.
                        with nc.named_scope("ft"):
                            nc.vector.tensor_tensor(
                                ftw[:, 0 : H // 2], P[:, 0 : H // 2],
                                P[:, H // 2 : H], op=OP.add,
                            )
                            w = H // 4
                            while w >= 2 * T:
                                nc.vector.tensor_tensor(
                                    ftw[:, 0:w], ftw[:, 0:w],
                                    ftw[:, w : 2 * w], op=OP.add,
                                )
                                w //= 2
                            if h == 0:
                                nc.vector.tensor_copy(
                                    ftf[:], ftw[:, 0 : 2 * T]
                                )
                            else:
                                nc.vector.tensor_copy(
                                    fth[:], ftw[:, 0 : 2 * T]
                                )
                                nc.vector.tensor_tensor(
                                    ftf[:], ftf[:], fth[:], op=OP.add
                                )
                        # fs: strided folds IN-PLACE inside P (ft already
                        # consumed P)
                        with nc.named_scope("fs"):
                            w = T // 2
                            while w >= 2:
                                nc.vector.tensor_tensor(
                                    P3[:, :, 0:w], P3[:, :, 0:w],
                                    P3[:, :, w : 2 * w], op=OP.add,
                                )
                                w //= 2
                            nc.vector.reduce_sum(
                                fs[:, sl], P3[:, :, 0:2], axis=AX.X
                            )

                with nc.named_scope("softmax"):
                    # f32 finish of ft partials: [C, 2T] -> [C, T]
                    nc.vector.tensor_tensor(
                        ft[:], ftf[:, 0:T], ftf[:, T : 2 * T], op=OP.add
                    )
                    if b != 0:
                        # shared-product divides (f32, tiny)
                        nc.vector.tensor_tensor(fs[:], fs[:], rwt[:], op=OP.mult)
                        nc.vector.tensor_tensor(ft[:], ft[:], rws[:], op=OP.mult)
                    # logits are tanh outputs in [-1,1]: no max-subtraction
                    # needed; exp's fused accum_out gives the softmax sum.
                    ssum = spool.tile([C, 1], F32, tag="ssum")
                    rec = spool.tile([C, 1], F32, tag="rec")
                    nc.scalar.activation(fs[:], fs[:], AF.Tanh)
                    nc.scalar.activation(
                        fs[:], fs[:], AF.Exp, accum_out=ssum[:, 0:1]
                    )
                    nc.vector.reciprocal(rec[:], ssum[:])
                    a2 = spool.tile([C, 2 * S], F16, tag="a2")
                    nc.vector.tensor_scalar(
                        out=a2.rearrange("p (s j) -> p s j", j=2),
                        in0=fs[:].unsqueeze(2).to_broadcast((C, S, 2)),
                        scalar1=rec[:, 0:1], scalar2=100.0,
                        op0=OP.mult, op1=OP.mult,
                    )

                    ssum2 = spool.tile([C, 1], F32, tag="ssum2")
                    rec2 = spool.tile([C, 1], F32, tag="rec2")
                    nc.scalar.activation(ft[:], ft[:], AF.Tanh)
                    nc.scalar.activation(
                        ft[:], ft[:], AF.Exp, accum_out=ssum2[:, 0:1]
                    )
                    nc.vector.reciprocal(rec2[:], ssum2[:])
                    g16 = spool.tile([C, T], F16, tag="g16")
                    nc.vector.tensor_scalar(
                        out=g16[:], in0=ft[:], scalar1=rec2[:, 0:1],
                        scalar2=None, op0=OP.mult,
                    )

                # final multiplies + store; eighths on the last batch so the
                # tail out-DMA exposure is short.  Output quarters live in
                # halves-sized work tiles (two store-pieces per tile).
                nf = 8 if b == B_LOC - 1 else 4
                SQ = S // nf
                g_bcq = g16.unsqueeze(1).to_broadcast((C, SQ, T))
                och = None
                for k in range(nf):
                    sl = slice(k * SQ, (k + 1) * SQ)
                    if k % (nf // 2) == 0:
                        och = ocpool.tile([C, H], F16, tag="oc", name=f"oc{b}_{k}")
                    o1 = och[
                        :,
                        (k % (nf // 2)) * SQ * T : (k % (nf // 2) + 1) * SQ * T,
                    ]
                    with nc.named_scope("final"):
                        o3 = o1.rearrange("p (s t) -> p s t", t=T)
                        nc.vector.tensor_tensor(
                            o3, X23[:, sl, :], g_bcq, op=OP.mult
                        )
                        # a-mul on fp16 pairs: innermost step-1 j keeps 2x
                        oP = o1.rearrange(
                            "p (s pr j) -> p s pr j", pr=T // 2, j=2
                        )
                        aP = (
                            a2[:, 2 * k * SQ : 2 * (k + 1) * SQ]
                            .rearrange("p (s j) -> p s j", j=2)
                            .unsqueeze(2)
                            .to_broadcast((C, SQ, T // 2, 2))
                        )
                        nc.vector.tensor_tensor(oP, oP, aP, op=OP.mult)
                        # alternate the two HWDGE rings so stores never queue
                        # behind each other on one ring
                        eng = nc.scalar if k % 2 == 0 else nc.sync
                        eng.dma_start(out[b, :, sl, :], o1)

    nc.compile()
    return nc


def get_nc():
    global _NC
    if _NC is None:
        _NC = build_nc()
    return _NC


def shard_inputs(x, Wspect, Wtemp):
    ws = np.ascontiguousarray(Wspect.reshape(C, T).astype(np.float16))
    wt = np.ascontiguousarray(Wtemp.reshape(C, S).astype(np.float16))
    x = np.ascontiguousarray(x.astype(np.float16))
    return [
        {"x": x[i * B_LOC : (i + 1) * B_LOC], "wspect": ws, "wtemp": wt}
        for i in range(N_CORES)
    ]


def unshard(results):
    return np.concatenate([r["out"] for r in results], axis=0).astype(np.float32)


def kernel(x, Wspect, Wtemp):
    nc = get_nc()
    in_maps = shard_inputs(x, Wspect, Wtemp)
    res = run_bass_kernel_spmd(nc, in_maps, core_ids=list(range(N_CORES)))
    return unshard(res.results)


# revision 15
# speedup vs baseline: 1.2515x; 1.2515x over previous
"""Trainium2 Bass kernel for nn_Attention_59528246723073.

Reference (per batch b, channel c; x[b,c] is [S=256, T=64]):
    fs = tanh(x @ Wspect[c])            # [S]   (contract T)
    ft = tanh(x.T @ Wtemp[c])           # [T]   (contract S)
    a  = softmax_S(fs) * 100
    g  = softmax_T(ft)
    out[b,c,s,t] = x[b,c,s,t] * a[s] * g[t]

Distribution: data-parallel over batch B=32 -> 4 per core on 8 cores.

All tensors are marshaled to fp16 on the host (input cast + output upcast
are host-side numpy), so every DMA is a plain HWDGE transfer and HBM
traffic is halved vs f32.  Per-core layout: [128 part = channels, S*T
free] fp16 per local batch; all big elementwise ops are DVE fp16
tensor_tensor in the 2x_1p perf mode (innermost step 1 everywhere),
issued at full FD wherever possible (per-instruction overhead dwarfs
the 58-cycle init, so fewer/bigger instructions win).

Shared-product trick: with wm[c,s,t] = Wtemp[c,s]*Wspect[c,t] built once
on chip, a single product P = x*wm serves BOTH reductions:
    fs[c,s] = (sum_t P[c,s,:]) / Wtemp[c,s]
    ft[c,t] = (sum_s P[c,:,t]) / Wspect[c,t]
which saves one full FD=16384 DVE pass per batch vs computing x*Wspect
and x*Wtemp separately.  The divides are tiny f32 ops on [C,S]/[C,T];
the exact same fp16 weight values are used in wm and in the reciprocals,
so the cancellation is clean (validated vs the f32 reference: rel err
1.05e-2 vs 0.98e-2 for the two-product scheme, gate 2e-2).

Per batch: P-mul; ft level-1 fold into a scratch tile (P kept intact),
in-place fp16 folds down to FD=2T, f32 finish; fs folds strided IN-PLACE
inside P (legal: ft has already consumed P); all DVE-serial so no
cross-engine semaphores on P.  Batch 0 is processed in graduated
s-chunks, with the wm build (ScalarE wt-replication + DVE muls)
interleaved chunk-by-chunk, so compute starts as soon as the first
piece of x and wm has landed.  Softmax skips the max-subtraction
(logits are tanh outputs in [-1,1]) and exp's fused accum_out provides
the sum.  Finals: g-mul (inner-contiguous bcast) then a-mul via
paired-duplicate a2[p,2s+j]=a[p,s], one oc tile per store piece (a
shared tile would add cross-engine WAR waits against the store DMA);
stores alternate between the two HWDGE rings; the last batch's finals
are split into eighths so the tail out-DMA exposure is short.
"""

import numpy as np

import concourse.tile as tile
from concourse import bacc, mybir
from concourse.bass_utils import run_bass_kernel_spmd

B, C, S, T = 32, 128, 256, 64
N_CORES = 8
B_LOC = B // N_CORES
F32 = mybir.dt.float32
F16 = mybir.dt.float16
H = S * T // 2  # 8192

_NC = None


def build_nc():
    nc = bacc.Bacc("TRN2", target_bir_lowering=False, debug=False)
    x = nc.dram_tensor("x", [B_LOC, C, S, T], F16, kind="ExternalInput")
    ws = nc.dram_tensor("wspect", [C, T], F16, kind="ExternalInput")
    wt = nc.dram_tensor("wtemp", [C, S], F16, kind="ExternalInput")
    out = nc.dram_tensor("out", [B_LOC, C, S, T], F16, kind="ExternalOutput")

    AF = mybir.ActivationFunctionType
    OP = mybir.AluOpType
    AX = mybir.AxisListType

    with tile.TileContext(nc) as tc:
        with (
            tc.tile_pool(name="consts", bufs=1) as cpool,
            tc.tile_pool(name="x2", bufs=2) as x2pool,
            tc.tile_pool(name="pp", bufs=1) as ppool,
            tc.tile_pool(name="scr", bufs=2) as scrpool,
            tc.tile_pool(name="wtr", bufs=2) as wtrpool,
            tc.tile_pool(name="ocp", bufs=4) as ocpool,
            tc.tile_pool(name="small", bufs=1) as spool,
        ):
            # --- constants (fp16 straight from HBM via HWDGE) ---
            ws16 = cpool.tile([C, T], F16)
            nc.sync.dma_start(ws16[:], ws[:])
            wt16 = cpool.tile([C, S], F16)
            nc.scalar.dma_start(wt16[:], wt[:])
            # f32 reciprocals of the fp16 weights (for the shared-product
            # divides)
            w32 = spool.tile([C, S], F32, tag="w32")
            rws = cpool.tile([C, T], F32)
            nc.vector.tensor_copy(w32[:, 0:T], ws16[:])
            nc.vector.reciprocal(rws[:], w32[:, 0:T])
            w32b = spool.tile([C, S], F32, tag="w32b")
            rwt = cpool.tile([C, S], F32)
            nc.vector.tensor_copy(w32b[:], wt16[:])
            nc.vector.reciprocal(rwt[:], w32b[:])

            # wm = wt_bcast * ws_bcast, built in graduated 32-row pieces that
            # interleave with batch 0's chunks.  Each piece: ScalarE
            # replicates wt along t into a small rotating scratch, DVE
            # multiplies by ws.
            wm = cpool.tile([C, S * T], F16)
            wm3 = wm.rearrange("p (s t) -> p s t", t=T)

            def build_wm(sl):
                for p0 in range(sl.start, sl.stop, 32):
                    wtr = wtrpool.tile(
                        [C, 32 * T], F16, tag="wtr", name=f"wtr{p0}"
                    )
                    wtr3 = wtr.rearrange("p (s t) -> p s t", t=T)
                    nc.scalar.activation(
                        wtr3,
                        wt16[:, p0 : p0 + 32]
                        .unsqueeze(2)
                        .to_broadcast((C, 32, T)),
                        AF.Copy,
                    )
                    nc.vector.tensor_tensor(
                        wm3[:, p0 : p0 + 32, :],
                        wtr3,
                        ws16.unsqueeze(1).to_broadcast((C, 32, T)),
                        op=OP.mult,
                    )

            for b in range(B_LOC):
                X2 = x2pool.tile([C, S * T], F16, tag="X2")
                X23 = X2.rearrange("p (s t) -> p s t", t=T)
                fs = spool.tile([C, S], F32, tag="fs")
                ft = spool.tile([C, T], F32, tag="ft")
                fth = spool.tile([C, 2 * T], F16, tag="fth")
                ftf = spool.tile([C, 2 * T], F32, tag="ftf")
                ftw = scrpool.tile([C, H], F16, tag="scr", name=f"ftw{b}")
                P = ppool.tile([C, S * T], F16, tag="P")
                P3 = P.rearrange("p (s t) -> p s t", t=T)

                # graduated chunks on the first batch (wm built piecewise,
                # just ahead of each chunk); single full-FD pass afterwards
                chunks = (32, 32, 64, 128) if b == 0 else (256,)
                s0 = 0
                for k, sc in enumerate(chunks):
                    sl = slice(s0, s0 + sc)
                    fsl = slice(s0 * T, (s0 + sc) * T)
                    if b == 0:
                        for q0 in range(s0, s0 + sc, 128):
                            build_wm(slice(q0, min(q0 + 128, s0 + sc)))
                    with nc.named_scope("load"):
                        for q0 in range(s0, s0 + sc, 64):
                            sq = slice(q0, min(q0 + 64, s0 + sc))
                            nc.sync.dma_start(
                                X2[:, sq.start * T : sq.stop * T],
                                x[b, :, sq, :],
                            )
                    with nc.named_scope("pmul"):
                        nc.vector.tensor_tensor(
                            P[:, fsl], X2[:, fsl], wm[:, fsl], op=OP.mult
                        )
                    # ft: level-1 fold into scratch (keeps P intact), then
                    # in-place fp16 folds down to FD=2T, f32 accumulate.
                    with nc.named_scope("ft"):
                        w = sc * T // 2
                        nc.vector.tensor_tensor(
                            ftw[:, 0:w], P[:, fsl.start : fsl.start + w],
                            P[:, fsl.start + w : fsl.stop], op=OP.add,
                        )
                        w //= 2
                        while w >= 2 * T:
                            nc.vector.tensor_tensor(
                                ftw[:, 0:w], ftw[:, 0:w], ftw[:, w : 2 * w],
                                op=OP.add,
                            )
                            w //= 2
                        if k == 0:
                            nc.vector.tensor_copy(ftf[:], ftw[:, 0 : 2 * T])
                        else:
                            nc.vector.tensor_copy(fth[:], ftw[:, 0 : 2 * T])
                            nc.vector.tensor_tensor(
                                ftf[:], ftf[:], fth[:], op=OP.add
                            )
                    # fs: strided folds IN-PLACE inside P (ft already read P)
                    with nc.named_scope("fs"):
                        p3c = P3[:, sl, :]
                        w = T // 2
                        while w >= 2:
                            nc.vector.tensor_tensor(
                                p3c[:, :, 0:w], p3c[:, :, 0:w],
                                p3c[:, :, w : 2 * w], op=OP.add,
                            )
                            w //= 2
                        nc.vector.reduce_sum(
                            fs[:, sl], p3c[:, :, 0:2], axis=AX.X
                        )
                    s0 += sc

                with nc.named_scope("softmax"):
                    # f32 finish of ft partials, then shared-product divides
                    nc.vector.tensor_tensor(
                        ft[:], ftf[:, 0:T], ftf[:, T : 2 * T], op=OP.add
                    )
                    nc.vector.tensor_tensor(fs[:], fs[:], rwt[:], op=OP.mult)
                    nc.vector.tensor_tensor(ft[:], ft[:], rws[:], op=OP.mult)
                    # logits are tanh outputs in [-1,1]: no max-subtraction
                    # needed; exp's fused accum_out gives the softmax sum.
                    ssum = spool.tile([C, 1], F32, tag="ssum")
                    rec = spool.tile([C, 1], F32, tag="rec")
                    nc.scalar.activation(fs[:], fs[:], AF.Tanh)
                    nc.scalar.activation(
                        fs[:], fs[:], AF.Exp, accum_out=ssum[:, 0:1]
                    )
                    nc.vector.reciprocal(rec[:], ssum[:])
                    a2 = spool.tile([C, 2 * S], F16, tag="a2")
                    nc.vector.tensor_scalar(
                        out=a2.rearrange("p (s j) -> p s j", j=2),
                        in0=fs[:].unsqueeze(2).to_broadcast((C, S, 2)),
                        scalar1=rec[:, 0:1], scalar2=100.0,
                        op0=OP.mult, op1=OP.mult,
                    )

                    ssum2 = spool.tile([C, 1], F32, tag="ssum2")
                    rec2 = spool.tile([C, 1], F32, tag="rec2")
                    nc.scalar.activation(ft[:], ft[:], AF.Tanh)
                    nc.scalar.activation(
                        ft[:], ft[:], AF.Exp, accum_out=ssum2[:, 0:1]
                    )
                    nc.vector.reciprocal(rec2[:], ssum2[:])
                    g16 = spool.tile([C, T], F16, tag="g16")
                    nc.vector.tensor_scalar(
                        out=g16[:], in0=ft[:], scalar1=rec2[:, 0:1],
                        scalar2=None, op0=OP.mult,
                    )

                # final multiplies + store; eighths on the last batch so the
                # tail out-DMA exposure is short.
                nf = 8 if b == B_LOC - 1 else 4
                SQ = S // nf
                g_bcq = g16.unsqueeze(1).to_broadcast((C, SQ, T))
                for k in range(nf):
                    sl = slice(k * SQ, (k + 1) * SQ)
                    with nc.named_scope("final"):
                        oc = ocpool.tile(
                            [C, SQ * T], F16, tag="oc", name=f"oc{b}_{k}"
                        )
                        o3 = oc.rearrange("p (s t) -> p s t", t=T)
                        nc.vector.tensor_tensor(
                            o3, X23[:, sl, :], g_bcq, op=OP.mult
                        )
                        # a-mul on fp16 pairs: innermost step-1 j keeps 2x
                        oP = oc.rearrange(
                            "p (s pr j) -> p s pr j", pr=T // 2, j=2
                        )
                        aP = (
                            a2[:, 2 * k * SQ : 2 * (k + 1) * SQ]
                            .rearrange("p (s j) -> p s j", j=2)
                            .unsqueeze(2)
                            .to_broadcast((C, SQ, T // 2, 2))
                        )
                        nc.vector.tensor_tensor(oP, oP, aP, op=OP.mult)
                        # alternate the two HWDGE rings so stores never queue
                        # behind each other on one ring
                        eng = nc.scalar if k % 2 == 0 else nc.sync
                        eng.dma_start(out[b, :, sl, :], oc[:])

    nc.compile()
    return nc


def get_nc():
    global _NC
    if _NC is None:
        _NC = build_nc()
    return _NC


def shard_inputs(x, Wspect, Wtemp):
    ws = np.ascontiguousarray(Wspect.reshape(C, T).astype(np.float16))
    wt = np.ascontiguousarray(Wtemp.reshape(C, S).astype(np.float16))
    x = np.ascontiguousarray(x.astype(np.float16))
    return [
        {"x": x[i * B_LOC : (i + 1) * B_LOC], "wspect": ws, "wtemp": wt}
        for i in range(N_CORES)
    ]


def unshard(results):
    return np.concatenate([r["out"] for r in results], axis=0).astype(np.float32)


def kernel(x, Wspect, Wtemp):
    nc = get_nc()
    in_maps = shard_inputs(x, Wspect, Wtemp)
    res = run_bass_kernel_spmd(nc, in_maps, core_ids=list(range(N_CORES)))
    return unshard(res.results)
